# revision 2
# baseline (speedup 1.0000x reference)
"""LocallyConnected1d (B=32, C=32, L=4096, K=7, stride=1) Trainium2 Bass kernel.

v7: bf16, shared stationary loads (3 PE instructions per position), batched
progress-semaphore writes, dual-ring DMA streaming.

Strategy (hardcoded for this problem):
  - Shard L_out=4090 across 8 cores (sequence parallel), 512 positions/core
    (padded; core 7 carries 6 zero-padded positions).
  - Host pre-permutes operands into PE-friendly bf16 layouts, chunked along
    the position axis (CHUNK=128 positions per chunk, 4 chunks):
      x2 chunk j [128, 132*32]: partition (tap-band kk 0..3, in_C i),
                                col (c, b) position-major, positions
                                [128j, 128j+132) (4-position halo for the
                                tail reuse)
      w1 chunk j [128, 32*128]: partition (kk, i), col (o, l_loc), taps 0..3
      w2 chunk j [ 96, 32*128]: partition (kk, i), col (o, l_loc), taps 4..6
    Chunk DMAs are interleaved (x2_j, w1_j, w2_j) and spread across both
    HWDGE rings (sync=SP, scalar=ACT) so completion latencies pipeline.
  - PE, per position l (col group cg = l%4):
      * paired matmul (32-col LDWEIGHTS kept): psum[b,o] of position l
        accumulates taps 0-3 -- stationary x[:, l] (K=128 = 4 bands x in_C),
        moving w1[:, (o, l)], start=True.
      * reuse matmul (generated LDWEIGHTS deleted): position l-4 accumulates
        taps 4-6 -- the just-loaded stationary at the same col group IS
        x[:, (l-4)+4], restricted to bands kk 0-2 (K=96), moving
        w2[:, (o, l-4)], stop=True.  (HW-probed: per-cg paired loads protect
        in-flight reuse matmuls from LDWEIGHTS pull-ahead.)
    Tail: positions 508-511 get their taps 4-6 from four normal paired
    matmuls on the chunk-3 halo columns.
  - PSUM: one 2 KB bank holds 64 positions (4 cgs x 16 slots x 32 out_C);
    banks ping-pong (bufs=2); VectorE drains a finished bank to SBUF
    (fp32->bf16, t-major: col = t*OC + o); output leaves in two 512 KB DMAs.
  - Post-passes: _delete_reuse_ldws, _split_matmul_waits, _thin_pe_incs.
"""

import sys

if "/opt/trn_rl_repo" not in sys.path:
    sys.path.insert(0, "/opt/trn_rl_repo")

import numpy as np
import ml_dtypes

import bass_rust
from concourse import bass, mybir, tile
from concourse.bass_utils import run_bass_kernel_spmd

_add_dep = bass_rust.add_dep_helper

# Problem constants (hardcoded; must match the grading reference).
B = 32          # batch
IC = 32         # in channels
L = 4096        # input length
OC = 32         # out channels
K = 7           # kernel taps
L_OUT = 4090    # (L - (K-1)) // 1

NCORES = 8
LP = 512        # positions per core (padded: 8*512 = 4096 >= 4090)
CHUNK = 128     # positions per DMA chunk
NCHUNK = LP // CHUNK
XCH = CHUNK + 4  # x2 chunk position extent (halo used by the tail)

XCCOLS = XCH * B         # x2 chunk per-partition cols: c*B + b
WCCOLS = OC * CHUNK      # w chunk tile cols: o*CHUNK + l_loc
OCOLS = (LP // 4) * OC   # out-stage per-partition cols: t*OC + o, t = l//4
WCOLS = NCHUNK * WCCOLS  # w dram tensor cols (chunk-major)
XCOLS = NCHUNK * XCCOLS  # x dram tensor cols (chunk-major, incl halo dup)

F32 = mybir.dt.float32
BF16 = mybir.dt.bfloat16
NPBF16 = ml_dtypes.bfloat16

_CACHE = {}


def _ap(t_ap, offset, dims):
    """Build a raw access pattern on the tensor behind an AP."""
    return bass_rust.AP(t_ap.tensor, int(offset), [[int(s), int(n)] for s, n in dims])


def _emit(reps=None):
    """Build the (identical-per-core) single-core program.

    reps: if set, wrap the whole body (DMAs included) in a hardware loop that
    executes it `reps` times -- used only for wall-clock timing calibration.
    """
    import contextlib

    nc = bass.Bass()
    x_d = nc.dram_tensor("x2", [128, XCOLS], BF16, kind="ExternalInput")
    w1_d = nc.dram_tensor("w1", [128, WCOLS], BF16, kind="ExternalInput")
    w2_d = nc.dram_tensor("w2", [96, WCOLS], BF16, kind="ExternalInput")
    o_d = nc.dram_tensor("out", [128, OCOLS], BF16, kind="ExternalOutput")

    drop = set()
    with tile.TileContext(nc) as tc:
        with (
            tc.tile_pool(name="persist", bufs=1) as persist,
            tc.tile_pool(name="x2pool", bufs=3) as x2pool,
            tc.tile_pool(name="w1pool", bufs=3) as w1pool,
            tc.tile_pool(name="w2pool", bufs=3) as w2pool,
            tc.tile_pool(name="psum", bufs=2, space=bass.MemorySpace.PSUM) as psum,
        ):
            ost = persist.tile([128, OCOLS], BF16, name="ostage")
            osa = ost[:]

            loop = (
                tc.For_i(0, reps, 1, hint_engines=(mybir.EngineType.PE,))
                if reps is not None else contextlib.nullcontext()
            )
            with loop:
                _emit_body(nc, osa, x_d, w1_d, w2_d, o_d,
                           x2pool, w1pool, w2pool, psum, drop)
    _delete_reuse_ldws(nc, drop)
    _split_matmul_waits(nc)
    _thin_pe_incs(nc)
    return nc


def _emit_body(nc, osa, x_d, w1_d, w2_d, o_d, x2pool, w1pool, w2pool, psum,
               drop):
    x2c = [None] * NCHUNK
    w1c = [None] * NCHUNK
    w2c = [None] * NCHUNK
    psb = [None, None]  # psum bank ping-pong (bufs=2)
    prev_mm = [None]

    def chain(mm):
        # The stationary-sharing contract needs the PE stream to keep the
        # emission order (reuse matmul immediately after its paired load at
        # the same col group); the Tile scheduler otherwise interleaves
        # independent banks.  A no-sync dependency edge pins the order
        # without semaphore cost (same engine => program order).
        if prev_mm[0] is not None:
            _add_dep(mm.ins, prev_mm[0].ins, sync=False,
                     reason="pin PE order for stationary sharing")
        prev_mm[0] = mm
        return mm

    def slot_ap(p):
        t, cg = divmod(p, 4)
        g, s = divmod(t, 16)
        return g, _ap(psb[g % 2], 32 * cg * 512 + s * 32, [[512, 32], [1, 32]])

    def drain(g):
        # bank g = positions [64g, 64g+64) = t slots [16g, 16g+16):
        # fp32 -> bf16, t-major: col = t*OC + o
        nc.vector.tensor_copy(
            _ap(osa, g * 16 * OC, [[OCOLS, 128], [OC, 16], [1, OC]]),
            _ap(psb[g % 2], 0, [[512, 128], [32, 16], [1, 32]]),
        )
        if g == 3:
            # first half of the output (t < 64) is complete: ship it
            nc.scalar.dma_start(
                _ap(o_d[:], 0, [[OCOLS, 128], [1, OCOLS // 2]]),
                _ap(osa, 0, [[OCOLS, 128], [1, OCOLS // 2]]),
            )

    for l in range(LP):
        j, l_loc = divmod(l, CHUNK)
        cg = l % 4

        if l_loc == 0:
            # interleaved chunk DMAs across both HWDGE rings
            x2t = x2pool.tile([128, XCCOLS], BF16, tag="x2c", name=f"x2c{j}")
            x2c[j] = x2t[:]
            nc.sync.dma_start(
                _ap(x2c[j], 0, [[XCCOLS, 128], [1, XCCOLS]]),
                _ap(x_d[:], j * XCCOLS, [[XCOLS, 128], [1, XCCOLS]]),
            )
            w1t = w1pool.tile([128, WCCOLS], BF16, tag="w1c", name=f"w1c{j}")
            w1c[j] = w1t[:]
            nc.scalar.dma_start(
                _ap(w1c[j], 0, [[WCCOLS, 128], [1, WCCOLS]]),
                _ap(w1_d[:], j * WCCOLS, [[WCOLS, 128], [1, WCCOLS]]),
            )
            w2t = w2pool.tile([128, WCCOLS], BF16, tag="w2c", name=f"w2c{j}")
            w2c[j] = w2t[:]
            eng = nc.sync if j % 2 == 0 else nc.scalar
            eng.dma_start(
                _ap(w2c[j], 0, [[WCCOLS, 96], [1, WCCOLS]]),
                _ap(w2_d[:], j * WCCOLS, [[WCOLS, 96], [1, WCCOLS]]),
            )

        if l % 64 == 0:
            g = l // 64
            pgt = psum.tile([128, 512], F32, tag="ps", name=f"ps{g}")
            psb[g % 2] = pgt[:]

        # paired matmul: position l, taps 0-3 (K=128), LDW kept.
        # start=True clears the has_written accumulate-bits for the WHOLE
        # (col group x bank) region, so only the first position per (bank,
        # cg) may set it -- later positions would wipe the bits of earlier
        # slots whose taps 4-6 haven't accumulated yet (their value would
        # then be overwritten, not accumulated).  First writes with
        # start=False overwrite anyway (has_written=0 after the clear).
        _, out1 = slot_ap(l)
        chain(nc.tensor.matmul(
            out1,
            _ap(x2c[j], l_loc * B, [[XCCOLS, 128], [1, 32]]),
            _ap(w1c[j], l_loc, [[WCCOLS, 128], [CHUNK, OC]]),
            start=(l % 64 < 4), stop=False,
            tile_position=(0, 32 * cg), skip_group_check=True,
        ))
        # reuse matmul: position l-4, taps 4-6 (K=96, same stationary), LDW
        # deleted by _delete_reuse_ldws
        if l >= 4:
            p = l - 4
            jp, l_locp = divmod(p, CHUNK)
            gp, out2 = slot_ap(p)
            mm = chain(nc.tensor.matmul(
                out2,
                _ap(x2c[j], l_loc * B, [[XCCOLS, 96], [1, 32]]),
                _ap(w2c[jp], l_locp, [[WCCOLS, 96], [CHUNK, OC]]),
                start=False, stop=True,
                tile_position=(0, 32 * cg), skip_group_check=True,
            ))
            drop.add(mm.ins.name)
            if p % 64 == 63:
                drain(gp)

    # tail: taps 4-6 of positions 508-511 from the chunk-3 halo columns
    for c in range(4):
        p = LP - 4 + c
        jp, l_locp = divmod(p, CHUNK)
        gp, out2 = slot_ap(p)
        chain(nc.tensor.matmul(
            out2,
            _ap(x2c[NCHUNK - 1], (CHUNK + c) * B, [[XCCOLS, 96], [1, 32]]),
            _ap(w2c[jp], l_locp, [[WCCOLS, 96], [CHUNK, OC]]),
            start=False, stop=True,
            tile_position=(0, 32 * c), skip_group_check=True,
        ))
    drain(7)

    nc.scalar.dma_start(
        _ap(o_d[:], OCOLS // 2, [[OCOLS, 128], [1, OCOLS // 2]]),
        _ap(osa, OCOLS // 2, [[OCOLS, 128], [1, OCOLS // 2]]),
    )


def _delete_reuse_ldws(nc, drop):
    """Each nc.tensor.matmul lowers to its own Ldweights+Matmult pair.  The
    reuse matmuls (in `drop`) use the stationary their preceding paired
    matmul just loaded at the same col group, so their generated Ldweights
    are redundant: delete them, migrating any waits onto the next kept PE
    instruction (PE executes in order)."""
    for f in nc.m.functions:
        for bb in f.blocks:
            insts = list(bb.instructions)
            out = []
            pend = []
            for idx, ins in enumerate(insts):
                if (ins.opcode == "Ldweights"
                        and idx + 1 < len(insts)
                        and insts[idx + 1].opcode == "Matmult"
                        and insts[idx + 1].name in drop):
                    si = ins.sync_info
                    if si is not None:
                        assert not si.on_update, (
                            f"deleted Ldweights {ins.name} has updates")
                        if si.on_wait:
                            pend.extend(si.on_wait)
                    del nc.inst_map[ins.name]
                    continue
                if pend and str(ins.engine) == "EngineType.PE":
                    si = ins.sync_info or mybir.SyncInfo(on_wait=[], on_update=[])
                    ins.sync_info = mybir.SyncInfo(
                        on_wait=list(si.on_wait or []) + pend,
                        on_update=list(si.on_update or []),
                    )
                    pend = []
                out.append(ins)
            assert not pend, "trailing waits from deleted Ldweights"
            bb.instructions = out


def _split_matmul_waits(nc):
    """This walrus build allows at most one sync wait per instruction.
    Relocate each multi-wait instruction's waits onto a chain of single-wait
    NoOps inserted just before it on the same engine -- program order makes
    this semantically identical."""
    for f in nc.m.functions:
        for bb in f.blocks:
            insts = list(bb.instructions)
            out = []
            changed = False
            for ins in insts:
                si = ins.sync_info
                if (si is not None and si.on_wait
                        and len(si.on_wait) >= 2):
                    for w in si.on_wait:
                        nop = mybir.InstNoOp(
                            name=nc.get_next_instruction_name(),
                            ins=[], outs=[],
                            sync_info=mybir.SyncInfo(
                                on_wait=[w], on_update=[]),
                            bass_nofuse=True,
                            engine=ins.engine,
                        )
                        nc.inst_map[nop.name] = nop
                        out.append(nop)
                    ins.sync_info = mybir.SyncInfo(
                        on_wait=[], on_update=list(si.on_update))
                    changed = True
                out.append(ins)
            if changed:
                bb.instructions = out


def _thin_pe_incs(nc, every=8):
    """PE instructions (Ldweights and Matmult here) each sem-inc the PE
    progress semaphore so consumers (DVE drains, pool-reuse guards) can wait
    on it; the EVT_SEM register write serializes at ~26 ns each, which at
    ~1500 updates is pure PE-queue overhead.  The PE executes in order, so
    batching the increments (as sem-add-imm) preserves the cumulative count
    at every flush point.  Flush on every `every`-th updater, on the last
    one, and exactly where the cumulative count hits a value some
    instruction waits for (so every wait is satisfied by the same
    instruction as before the pass)."""
    for f in nc.m.functions:
        sem_ids = set()
        upds = []
        for bb in f.blocks:
            for ins in bb.instructions:
                si = ins.sync_info
                if si and si.on_update:
                    for u in si.on_update:
                        if (u.sync_type == "semaphore"
                                and u.update_mode == "sem-inc"
                                and (u.ant_name or "").startswith("PE")
                                and str(ins.engine) == "EngineType.PE"):
                            sem_ids.add(u.id)
                            upds.append((ins, u))
        if not upds:
            continue
        required = set()
        for bb in f.blocks:
            for ins in bb.instructions:
                si = ins.sync_info
                if si and si.on_wait:
                    for w in si.on_wait:
                        if w.id in sem_ids:
                            required.add(w.wait_value)
        n = len(upds)
        pending = 0
        cum = 0
        for i, (ins, u) in enumerate(upds, start=1):
            si = ins.sync_info
            pending += u.update_value
            cum += u.update_value
            if i % every == 0 or i == n or cum in required:
                u.update_value = pending
                if pending != 1:
                    u.update_mode = "sem-add-imm"
                pending = 0
                keep_u = True
            else:
                keep_u = False
            new_upd = [x for x in (si.on_update or [])
                       if not (x.sync_type == "semaphore" and x.id in sem_ids)]
            if keep_u:
                new_upd.append(u)
            ins.sync_info = mybir.SyncInfo(
                on_wait=list(si.on_wait or []), on_update=new_upd)


def _get_nc():
    if "nc" not in _CACHE:
        _CACHE["nc"] = _emit()
    return _CACHE["nc"]


def _shard_inputs(x, weight):
    """Pre-permute full inputs into the per-core kernel layouts (bf16)."""
    x = np.asarray(x, dtype=np.float32)
    weight = np.asarray(weight, dtype=np.float32)
    xpad = np.zeros((B, IC, NCORES * LP + XCH + 4), dtype=np.float32)
    xpad[:, :, :L] = x
    wpad = np.zeros((OC, IC, NCORES * LP, K), dtype=np.float32)
    wpad[:, :, :L_OUT, :] = weight

    in_maps = []
    for m in range(NCORES):
        l0 = m * LP
        # x2: (chunk j, band kk, i, c, b) with value xpad[b, i, l0+128j+c+kk]
        x2 = np.empty((NCHUNK, 4, IC, XCH, B), dtype=np.float32)
        for j in range(NCHUNK):
            win = xpad[:, :, l0 + j * CHUNK : l0 + j * CHUNK + XCH + 3]
            for kk in range(4):
                # (B, IC, XCH) -> (IC, XCH, B)
                x2[j, kk] = win[:, :, kk : kk + XCH].transpose(1, 2, 0)
        ws = wpad[:, :, l0 : l0 + LP, :]        # (OC, IC, LP, K)
        wt = ws.transpose(3, 1, 0, 2)           # (K, IC, OC, LP)
        # chunk-major columns: (NCHUNK, OC, CHUNK)
        wt = wt.reshape(K, IC, OC, NCHUNK, CHUNK).transpose(0, 1, 3, 2, 4)
        # dram layout [128, XCOLS]: partition (kk, i), col (j, c, b)
        x2d = x2.transpose(1, 2, 0, 3, 4)
        in_maps.append({
            "x2": np.ascontiguousarray(x2d).reshape(128, XCOLS).astype(NPBF16),
            "w1": np.ascontiguousarray(wt[0:4]).reshape(128, WCOLS).astype(NPBF16),
            "w2": np.ascontiguousarray(wt[4:7]).reshape(96, WCOLS).astype(NPBF16),
        })
    return in_maps


def _unshard_output(res):
    """res: list of per-core {"out": (128, OCOLS)} -> full (B, OC, L_OUT)."""
    out = np.empty((B, OC, NCORES * LP), dtype=np.float32)
    for m in range(NCORES):
        # partition (cg, b), col (t, o)
        arr = res[m]["out"].astype(np.float32).reshape(4, B, LP // 4, OC)
        out[:, :, m * LP : (m + 1) * LP] = (
            arr.transpose(1, 3, 2, 0).reshape(B, OC, LP)
        )
    return np.ascontiguousarray(out[:, :, :L_OUT])


def kernel(x, weight):
    nc = _get_nc()
    in_maps = _shard_inputs(x, weight)
    res = run_bass_kernel_spmd(nc, in_maps, list(range(NCORES))).results
    return _unshard_output(res)
